# revision 64
# baseline (speedup 1.0000x reference)
"""Trainium2 Bass kernel for the slot-attention-style loss (nn_LossFunctions_86397562126683).

Strategy: pure data parallel over 8 NeuronCores (batch 8192 -> 1024/core),
gamma replicated; each core computes a partial scalar loss, host sums the 8
partials.

Per-core pipeline (B=1024, S=7, D=128):
  - phase 0: KL (mu^2, exp(lv), sum lv) and mask entropy on whole-core
    [128, 7168] tiles -- exp/ln batched so the ACT table set loads once;
    mask_term and 1/mask precomputed for all chunks
  - 8 batch-chunks of 128: ai/o/ah loaded block-wise ([112=(16b x 7slot),
    8 blocks, 128]), ah also naturally (reorder, r, norms); 16-batch
    block-diagonal Gram matmuls on PE in bf16 (operands via PE transposes /
    xbar DMA transposes); cross terms masked + strided-segment reduce; slot
    norms ride along; SWDGE DMAs repartition to natural [b, .] layout
  - SQ = ni + nh - 2G, D = sqrt(relu(SQ)), W = |0.5*SQ2 - gamma| * mask_term
    built for both assignments into one [128, 2, 98] tile; two PE transposes
    feed [49, 128] dm/val lanes
  - permutation totals: D-side f32 matmuls (exact equality for the eq
    extraction), V-side f32r matmuls, against meet-in-the-middle tables
    (210 ordered triples + 840 ordered quads in one 35x24 block); TA+VA
    share one PSUM bank, late-read banks evicted to SBUF to unblock the
    next assignment's matmuls
  - segmented mins + equality-mask extraction recover the optimal
    assignment value sums without materializing all 5040 permutation totals
  - elementwise work spread across DVE/ACT/Pool; final partition sum via
    ones-matmul
"""

import itertools
import os
import sys

import numpy as np

sys.path.insert(0, "/opt/trn_rl_repo")

BATCH = 8192
N_CORES = 8
B = BATCH // N_CORES          # 1024 per core
S = 7
D = 128
NBH = B // 128                # 8 chunks of 128
BETA = 4.0

_nc_cache = {}


def _build_tables():
    s_sets = list(itertools.combinations(range(S), 3))  # 35, lex order
    MT = np.zeros((49, 210), np.float32)
    MB = np.zeros((49, 840), np.float32)
    for si, sset in enumerate(s_sets):
        for k, perm in enumerate(itertools.permutations(sset)):
            for i, j in enumerate(perm):
                MT[i * 7 + j, si * 6 + k] = 1.0
        quad = tuple(sorted(set(range(S)) - set(sset)))
        for k, perm in enumerate(itertools.permutations(quad)):
            for i2, j in enumerate(perm):
                MB[(3 + i2) * 7 + j, si * 24 + k] = 1.0
    return MT, MB


def _build_extmask():
    # Gram block output: partition p=(g,i) [p=g*7+i], free n=(g',j)
    # [n=g'*7+j].  Keep only matching batch lanes g' == g.
    m = np.zeros((112, 112), np.float32)
    for p in range(112):
        for n in range(112):
            if n // 7 == p // 7:
                m[p, n] = 1.0
    return m


def build_bass():
    import contextlib

    import concourse.bacc as bacc
    import concourse.bass as bass
    import concourse.tile as tile
    from concourse import mybir
    from concourse.masks import make_identity

    f32 = mybir.dt.float32
    f32r = mybir.dt.float32r
    bf16 = mybir.dt.bfloat16
    Alu = mybir.AluOpType
    Act = mybir.ActivationFunctionType
    AX = mybir.AxisListType

    MT_np, MB_np = _build_tables()
    EXT_np = _build_extmask()
    # K-stacked A-side weights: rows 0-48 (dm) -> D-totals cols 0-209,
    # rows 49-97 (val) -> V-totals cols 210-419

    nc = bacc.Bacc(
        "TRN2",
        target_bir_lowering=False,
        debug=False,
        enable_asserts=False,
        num_devices=N_CORES,
    )

    ai_d = nc.dram_tensor("ai", [B, S, D], f32, kind="ExternalInput").ap()
    ah_d = nc.dram_tensor("a_hat", [B, S, D], f32, kind="ExternalInput").ap()
    mu_d = nc.dram_tensor("mu_q", [B, S, D], f32, kind="ExternalInput").ap()
    lv_d = nc.dram_tensor("logvar_q", [B, S, D], f32, kind="ExternalInput").ap()
    o_d = nc.dram_tensor("o", [B, S, D], f32, kind="ExternalInput").ap()
    mask_d = nc.dram_tensor("learned_mask", [B, 1, D], f32, kind="ExternalInput").ap()
    gam_d = nc.dram_tensor("gamma", [S * D], f32, kind="ExternalInput").ap()
    out_d = nc.dram_tensor("out", [1, 1], f32, kind="ExternalOutput").ap()

    mt_d = nc.inline_tensor(MT_np, "mt_const").ap()
    mb_d = nc.inline_tensor(MB_np, "mb_const").ap()
    mtr_d = nc.inline_tensor(MT_np, "mtr_const").ap()
    mbr_d = nc.inline_tensor(MB_np, "mbr_const").ap()
    ext_d = nc.inline_tensor(EXT_np, "ext_const").ap()

    with tile.TileContext(nc) as tc:
        ctx = contextlib.ExitStack()
        with ctx:
            consts = ctx.enter_context(tc.tile_pool(name="consts", bufs=1))
            state = ctx.enter_context(tc.tile_pool(name="state", bufs=1))
            pnat = ctx.enter_context(tc.tile_pool(name="nat", bufs=2))
            psmall = ctx.enter_context(tc.tile_pool(name="small", bufs=3))
            pwork = ctx.enter_context(tc.tile_pool(name="work", bufs=2))
            pcast = ctx.enter_context(tc.tile_pool(name="cast", bufs=2))
            ptr = ctx.enter_context(tc.tile_pool(name="transp", bufs=2))
            pg = ctx.enter_context(tc.tile_pool(name="gpsum", bufs=1, space="PSUM"))
            ptp = ctx.enter_context(tc.tile_pool(name="tppsum", bufs=1, space="PSUM"))
            pt = ctx.enter_context(tc.tile_pool(name="trpsum", bufs=1, space="PSUM"))
            ptav = ctx.enter_context(tc.tile_pool(name="tavpsum", bufs=1, space="PSUM"))
            ptb = ctx.enter_context(tc.tile_pool(name="tbpsum", bufs=1, space="PSUM"))
            pvb = ctx.enter_context(tc.tile_pool(name="vbpsum", bufs=1, space="PSUM"))

            # ---- constants -------------------------------------------------
            mt_c = consts.tile([49, 210], f32, tag="mt")
            mb_c = consts.tile([49, 840], f32, tag="mb")
            mtr_c = consts.tile([49, 210], f32r, tag="mtr")
            mbr_c = consts.tile([49, 840], f32r, tag="mbr")
            ext_c = consts.tile([112, 112], f32, tag="ext")
            identb = consts.tile([128, 128], bf16, tag="identb")
            ident = consts.tile([128, 128], f32, tag="ident")
            ones_c = consts.tile([128, 1], f32, tag="ones")
            gam7 = consts.tile([128, 7], f32, tag="gam7")
            nc.sync.dma_start(out=mt_c, in_=mt_d)
            nc.sync.dma_start(out=mb_c, in_=mb_d)
            nc.gpsimd.dma_start(out=mtr_c, in_=mtr_d)
            nc.gpsimd.dma_start(out=mbr_c, in_=mbr_d)
            nc.sync.dma_start(out=ext_c, in_=ext_d)
            make_identity(nc, identb)
            make_identity(nc, ident)
            nc.vector.memset(ones_c, 1.0)
            gam_b = bass.AP(tensor=gam_d.tensor, offset=0, ap=[[0, 128], [1, 7]])
            nc.sync.dma_start(out=gam7, in_=gam_b)
            eps_c = consts.tile([128, 1], f32, tag="eps")
            nc.vector.memset(eps_c, 1e-10)

            # ---- persistent accumulators ----------------------------------
            REC = [
                state.tile([128, NBH * 2], f32, tag=f"rec{a}", name=f"rec{a}")
                for a in range(2)
            ]
            KLA = state.tile([128, 4], f32, tag="kla")
            KLB = state.tile([128, 4], f32, tag="klb")
            KLC = state.tile([128, 1], f32, tag="klc")
            ENT = state.tile([128, 1], f32, tag="ent")
            REO = state.tile([128, NBH], f32, tag="reo")

            ai_f = ai_d.flatten_outer_dims()   # [7168, 128] rows b*7+i
            ah_f = ah_d.flatten_outer_dims()
            o_f = o_d.flatten_outer_dims()

            # ---- phase 0: KL + entropy on whole-core tiles ----------------
            # (exp/ln grouped here so the act table set switches once; the
            # chunk loop below then only ever needs the sqrt set + fillers)
            pbig = ctx.enter_context(tc.tile_pool(name="big", bufs=1))
            mu_t = pbig.tile([128, NBH, S * D], f32, tag="bigin", name="mu_t")
            nc.gpsimd.dma_start(
                out=mu_t,
                in_=bass.AP(
                    tensor=mu_d.tensor, offset=mu_d.offset,
                    ap=[[S * D, 128], [128 * S * D, NBH], [1, S * D]],
                ),
            )
            H = NBH // 4
            for h in range(4):
                sq_t = pbig.tile([128, H, S * D], f32, tag="bigsq", name=f"sq{h}")
                nc.scalar.activation(
                    out=sq_t, in_=mu_t[:, h * H : (h + 1) * H, :],
                    func=Act.Square, accum_out=KLA[:, h : h + 1],
                )
            lv_t = pbig.tile([128, NBH, S * D], f32, tag="bigin", name="lv_t")
            nc.gpsimd.dma_start(
                out=lv_t,
                in_=bass.AP(
                    tensor=lv_d.tensor, offset=lv_d.offset,
                    ap=[[S * D, 128], [128 * S * D, NBH], [1, S * D]],
                ),
            )
            for h in range(4):
                el_t = pbig.tile([128, H, S * D], f32, tag="bigsq", name=f"el{h}")
                nc.scalar.activation(
                    out=el_t, in_=lv_t[:, h * H : (h + 1) * H, :],
                    func=Act.Exp, accum_out=KLB[:, h : h + 1],
                )
            KLC2 = state.tile([128, 4], f32, tag="klc2")
            for h in range(4):
                il_t = pbig.tile([128, H, S * D], f32, tag="bigsq", name=f"il{h}")
                nc.scalar.activation(
                    out=il_t, in_=lv_t[:, h * H : (h + 1) * H, :],
                    func=Act.Identity, accum_out=KLC2[:, h : h + 1],
                )
            nc.vector.tensor_reduce(out=KLC, in_=KLC2, axis=AX.X, op=Alu.add)

            mask_t = state.tile([128, NBH, D], f32, tag="maskt")
            nc.gpsimd.dma_start(
                out=mask_t,
                in_=bass.AP(
                    tensor=mask_d.tensor, offset=mask_d.offset,
                    ap=[[D, 128], [128 * D, NBH], [1, D]],
                ),
            )
            lnm_t = state.tile([128, NBH, D], f32, tag="lnmt")
            nc.scalar.activation(out=lnm_t, in_=mask_t, func=Act.Ln, bias=eps_c)
            jm_t = state.tile([128, NBH, D], f32, tag="jmt")
            nc.vector.scalar_tensor_tensor(
                out=jm_t, in0=lnm_t, scalar=1.0, in1=mask_t,
                op0=Alu.mult, op1=Alu.mult, accum_out=ENT,
            )
            summask = state.tile([128, NBH], f32, tag="summask")
            nc.vector.tensor_reduce(out=summask, in_=mask_t, axis=AX.X, op=Alu.add)
            mts_all = state.tile([128, NBH], f32, tag="mtsall")
            nc.vector.tensor_scalar(
                out=mts_all, in0=summask, scalar1=-1.0, scalar2=float(D),
                op0=Alu.mult, op1=Alu.add,
            )

            for bh in range(NBH):
                sl = slice(bh * 128, (bh + 1) * 128)

                # block-layout loads: partition p=(g,i) [p=g*7+i], free (m, d)
                ai_b = pnat.tile([112, 8, D], f32, tag="ai")
                ah_b = pnat.tile([112, 8, D], f32, tag="ahb")
                o_b = pnat.tile([112, 8, D], f32, tag="o")
                for t_blk, t_dram in ((ai_b, ai_f), (ah_b, ah_f), (o_b, o_f)):
                    src = bass.AP(
                        tensor=t_dram.tensor,
                        offset=t_dram.offset + bh * 128 * S * D,
                        ap=[[S * D, 16], [D, S], [16 * S * D, 8], [1, D]],
                    )
                    nc.sync.dma_start(out=t_blk, in_=src)

                ah_n = pnat.tile([128, S, D], f32, tag="ah")
                nc.sync.dma_start(out=ah_n, in_=ah_d[sl])
                mask_n = mask_t[:, bh, :]

                recip = psmall.tile([128, D], f32, tag="recip")
                nc.vector.reciprocal(out=recip, in_=mask_n)
                recb = psmall.tile([128, D], bf16, tag="recb")
                nc.gpsimd.tensor_copy(out=recb, in_=recip)

                # r natural (norms only)
                r_n = pnat.tile([128, S, D], f32, tag="r")
                recip_bc = recip.unsqueeze(1).broadcast_to([128, S, D])
                nc.gpsimd.tensor_tensor(out=r_n, in0=ah_n, in1=recip_bc, op=Alu.mult)

                # mask_term = 128 - sum_d(mask), from phase 0
                mts = mts_all[:, bh : bh + 1]

                # ---- natural norms: ah, r ---------------------------------
                norms = {}
                for nm, src_n in (("ah", ah_n), ("r", r_n)):
                    sq = pwork.tile([128, S, D], f32, tag="sq")
                    nc.scalar.square(out=sq, in_=src_n)
                    nrm = psmall.tile([128, S], f32, tag=f"n_{nm}")
                    nc.vector.tensor_reduce(out=nrm, in_=sq, axis=AX.X, op=Alu.add)
                    norms[nm] = nrm

                # block norms of ai/o ride along in gext col 7 (below)
                sqa = pwork.tile([112, 8, D], f32, tag="sqa")
                nc.gpsimd.tensor_tensor(out=sqa, in0=ai_b, in1=ai_b, op=Alu.mult)
                sqo = pwork.tile([112, 8, D], f32, tag="sqo")
                nc.scalar.square(out=sqo, in_=o_b)

                # ---- bf16 casts -------------------------------------------
                aib = pcast.tile([112, 8, D], bf16, tag="aib")
                ahb = pcast.tile([112, 8, D], bf16, tag="ahbb")
                ob = pcast.tile([112, 8, D], bf16, tag="ob")
                nc.gpsimd.tensor_copy(out=aib, in_=ai_b)
                nc.gpsimd.tensor_copy(out=ahb, in_=ah_b)
                nc.scalar.copy(out=ob, in_=o_b)

                # ---- transposed operands ----------------------------------
                # ai/o/ah + recip via xbar DMA transpose, spread across the
                # SP/ACT/Pool/PE DMA queues. tt_*: [128 d, (m, 112=(g,slot))]
                tT = {}
                for nm, src_bf in (("ai", aib), ("o", ob)):
                    tps = ptp.tile([128, 8, 112], bf16, tag="tps", name="tps")
                    for m in range(8):
                        nc.tensor.transpose(
                            tps[:, m, :], src_bf[:, m, :], identb[0:112, 0:112]
                        )
                    tt = pcast.tile([128, 8, 112], bf16, tag=f"t_{nm}")
                    nc.scalar.copy(out=tt, in_=tps)
                    tT[nm] = tt
                tt_ah = pcast.tile([128, 8, 112], bf16, tag="t_ah")
                for m in range(8):
                    eng = (nc.sync, nc.scalar)[m % 2]
                    eng.dma_start_transpose(tt_ah[:, m, :], ahb[:, m, :])
                tT["ah"] = tt_ah
                recT = psmall.tile([128, D], bf16, tag="recT")
                nc.sync.dma_start_transpose(recT, recb)
                # r transposed: tt_r = tt_ah * recT (broadcast over slot)
                tt_r = pcast.tile([128, 8, 112], bf16, tag="t_r")
                rec_bc = bass.AP(
                    tensor=recT.tensor, offset=recT.offset,
                    ap=[recT.ap[0], [16, 8], [1, 16], [0, 7]],
                )
                ttah_v = bass.AP(
                    tensor=tt_ah.tensor, offset=tt_ah.offset,
                    ap=[tt_ah.ap[0], [112, 8], [7, 16], [1, 7]],
                )
                ttr_v = bass.AP(
                    tensor=tt_r.tensor, offset=tt_r.offset,
                    ap=[tt_r.ap[0], [112, 8], [7, 16], [1, 7]],
                )
                nc.vector.tensor_tensor(out=ttr_v, in0=ttah_v, in1=rec_bc, op=Alu.mult)
                tT["r"] = tt_r

                # ---- two assignments --------------------------------------
                # phase A: grams + extraction for BOTH assignments into
                # one merged gext (asg a occupies cols a*8 .. a*8+8)
                gext = pwork.tile([112, 8, 16], f32, tag="gext")
                for asg, (lt, rt, sq_blk) in enumerate(
                    (("ai", "ah", sqa), ("o", "r", sqo))
                ):
                    LT, RT = tT[lt], tT[rt]
                    # lhs-tensor slot norms ride in col 7
                    nc.vector.tensor_reduce(
                        out=gext[:, :, asg * 8 + 7 : asg * 8 + 8],
                        in_=sq_blk, axis=AX.X, op=Alu.add,
                    )
                    for h in range(2):
                        gps = pg.tile([112, 4, 112], f32, tag="gram", name="gps")
                        for m4 in range(4):
                            m = h * 4 + m4
                            nc.tensor.matmul(
                                gps[:, m4, :], LT[:, m, :], RT[:, m, :]
                            )
                        mprod = pwork.tile([112, 4, 112], f32, tag="mprod")
                        ext_b = ext_c.unsqueeze(1).broadcast_to([112, 4, 112])
                        nc.vector.tensor_tensor(
                            out=mprod, in0=gps, in1=ext_b, op=Alu.mult
                        )
                        mp_r = bass.AP(
                            tensor=mprod.tensor, offset=mprod.offset,
                            ap=[mprod.ap[0], [112, 4], [1, 7], [7, 16]],
                        )
                        gext_g = bass.AP(
                            tensor=gext.tensor,
                            offset=gext.offset + asg * 8 + h * 4 * 16,
                            ap=[gext.ap[0], [16, 4], [1, 7]],
                        )
                        nc.vector.tensor_reduce(
                            out=gext_g, in_=mp_r, axis=AX.X, op=Alu.add
                        )

                # repartition to natural [128, (7 i, 16 c)] once for both
                # assignments, split across SWDGE and HWDGE queues
                gnat = pwork.tile([128, S, 16], f32, tag="gnat")
                for m in range(8):
                    eng = (nc.gpsimd, nc.sync)[m % 2]
                    eng.dma_start(
                        out=gnat[16 * m : 16 * (m + 1), :, :],
                        in_=gext[:, m, :],
                    )

                # ---- build dm/val feeds for both assignments (bf16) -------
                # fd[:, a, 0:49] = dm_a, fd[:, a, 49:98] = val_a
                fd = pwork.tile([128, 2, 98], f32, tag="fd")
                for asg, nat_nrm in enumerate(("ah", "r")):
                    base = asg * 8
                    g_ap = gnat[:, :, base : base + 7]
                    nl_ap = bass.AP(
                        tensor=gnat.tensor, offset=gnat.offset + base + 7,
                        ap=[gnat.ap[0], [16, 7], [0, 7]],
                    )
                    nr_t = norms[nat_nrm]
                    nr_ap = bass.AP(
                        tensor=nr_t.tensor, offset=nr_t.offset,
                        ap=[nr_t.ap[0], [0, 7], [1, 7]],
                    )
                    nsum = pwork.tile([128, 49], f32, tag="nsum")
                    nc.gpsimd.tensor_tensor(out=nsum, in0=nl_ap, in1=nr_ap, op=Alu.add)
                    sqm = pwork.tile([128, 49], f32, tag="sqm")
                    nc.vector.scalar_tensor_tensor(
                        out=sqm, in0=g_ap, scalar=-2.0, in1=nsum,
                        op0=Alu.mult, op1=Alu.add,
                    )
                    relu = pwork.tile([128, 49], f32, tag="relu")
                    nc.scalar.activation(out=relu, in_=sqm, func=Act.Relu)
                    nc.scalar.sqrt(out=fd[:, asg, 0:49], in_=relu)

                    if asg == 0:
                        nc.gpsimd.tensor_copy(out=fd[:, 0, 49:98], in_=sqm)
                    else:
                        gam_b49 = bass.AP(
                            tensor=gam7.tensor, offset=gam7.offset,
                            ap=[gam7.ap[0], [0, 7], [1, 7]],
                        )
                        w1 = pwork.tile([128, 49], f32, tag="w1")
                        nc.vector.scalar_tensor_tensor(
                            out=w1, in0=sqm, scalar=0.5, in1=gam_b49,
                            op0=Alu.mult, op1=Alu.subtract,
                        )
                        w2 = pwork.tile([128, 49], f32, tag="w2")
                        nc.scalar.activation(out=w2, in_=w1, func=Act.Abs)
                        nc.vector.tensor_scalar(
                            out=fd[:, 1, 49:98], in0=w2, scalar1=mts, scalar2=None,
                            op0=Alu.mult,
                        )

                # ---- transpose dm/val to [49, 128] feeds ------------------
                tpb = pt.tile([49, 4, 128], f32, tag="tpb", name="tpb")
                tpd = tpb[:, 0:2, :]
                tpv = tpb[:, 2:4, :]
                for a in range(2):
                    nc.tensor.transpose(tpd[:, a, :], fd[:, a, 0:49], ident)
                    nc.tensor.transpose(tpv[:, a, :], fd[:, a, 49:98], ident)
                feedsd = ptr.tile([49, 2, 128], f32, tag="feedsd")
                feedsv = ptr.tile([49, 2, 128], f32r, tag="feedsv")
                nc.scalar.copy(out=feedsd, in_=tpd)
                nc.scalar.copy(out=feedsv, in_=tpv)

                for asg in range(2):
                    dT = feedsd[:, asg, :]
                    vT = feedsv[:, asg, :]

                    # ---- totals matmuls + consumption ---------------------
                    # TAV: cols 0-209 = D A-totals, 210-419 = V A-totals
                    TAV = ptav.tile([128, 420], f32, tag="tav", name="TAV")
                    TB = ptb.tile([128, 840], f32, tag="tb", name="TB")
                    VB = pvb.tile([128, 840], f32, tag="vb", name="VB")
                    nc.tensor.matmul(TAV[:, 0:210], dT, mt_c)
                    nc.tensor.matmul(TAV[:, 210:420], vT, mtr_c)
                    nc.tensor.matmul(TB[:, 0:512], dT, mb_c[:, 0:512])
                    nc.tensor.matmul(TB[:, 512:840], dT, mb_c[:, 512:840])
                    nc.tensor.matmul(VB[:, 0:512], vT, mbr_c[:, 0:512])
                    nc.tensor.matmul(VB[:, 512:840], vT, mbr_c[:, 512:840])

                    # evict the late-read banks (TAV: eqA/jka, VB: jkb) to
                    # SBUF so those PSUM banks free early; TB's last PSUM
                    # reader (eqB) is mid-chain already
                    tavS = pwork.tile([128, 420], f32, tag="tavS")
                    tbS = pwork.tile([128, 840], f32, tag="tbS")
                    vbS = pwork.tile([128, 840], f32, tag="vbS")
                    nc.scalar.copy(out=tavS, in_=TAV)
                    nc.scalar.copy(out=tbS, in_=TB)
                    nc.scalar.copy(out=vbS, in_=VB)

                    A35 = pwork.tile([128, 35], f32, tag="a35")
                    ta_r = bass.AP(
                        tensor=tavS.tensor, offset=tavS.offset,
                        ap=[tavS.ap[0], [6, 35], [1, 6]],
                    )
                    nc.vector.tensor_reduce(out=A35, in_=ta_r, axis=AX.X, op=Alu.min)
                    B35 = pwork.tile([128, 35], f32, tag="b35")
                    tb_r = bass.AP(
                        tensor=TB.tensor, offset=TB.offset,
                        ap=[TB.ap[0], [24, 35], [1, 24]],
                    )
                    nc.vector.tensor_reduce(out=B35, in_=tb_r, axis=AX.X, op=Alu.min)

                    t35 = pwork.tile([128, 35], f32, tag="t35")
                    nc.gpsimd.tensor_tensor(out=t35, in0=A35, in1=B35, op=Alu.add)
                    minv = psmall.tile([128, 1], f32, tag="minv")
                    nc.vector.tensor_reduce(out=minv, in_=t35, axis=AX.X, op=Alu.min)
                    soh = pwork.tile([128, 35], f32, tag="soh")
                    nc.vector.tensor_scalar(
                        out=soh, in0=t35, scalar1=minv, scalar2=None, op0=Alu.is_equal
                    )
                    j35 = pwork.tile([128, 35], f32, tag="j35")
                    minTs = psmall.tile([128, 1], f32, tag="mint")
                    nc.vector.scalar_tensor_tensor(
                        out=j35, in0=A35, scalar=1.0, in1=soh,
                        op0=Alu.mult, op1=Alu.mult, accum_out=minTs,
                    )
                    j35b = pwork.tile([128, 35], f32, tag="j35b")
                    minBs = psmall.tile([128, 1], f32, tag="minb")
                    nc.vector.scalar_tensor_tensor(
                        out=j35b, in0=B35, scalar=1.0, in1=soh,
                        op0=Alu.mult, op1=Alu.mult, accum_out=minBs,
                    )

                    eqA = pwork.tile([128, 210], f32, tag="eqa")
                    nc.gpsimd.tensor_scalar(
                        out=eqA, in0=tavS[:, 0:210], scalar1=minTs, scalar2=None,
                        op0=Alu.is_equal,
                    )
                    eqB = pwork.tile([128, 840], f32, tag="eqb")
                    nc.gpsimd.tensor_scalar(
                        out=eqB, in0=tbS, scalar1=minBs, scalar2=None,
                        op0=Alu.is_equal,
                    )

                    jka = pwork.tile([128, 210], f32, tag="jka")
                    nc.vector.scalar_tensor_tensor(
                        out=jka, in0=tavS[:, 210:420], scalar=1.0, in1=eqA,
                        op0=Alu.mult, op1=Alu.mult,
                        accum_out=REC[asg][:, bh * 2 : bh * 2 + 1],
                    )
                    jkb = pwork.tile([128, 840], f32, tag="jkb")
                    nc.vector.scalar_tensor_tensor(
                        out=jkb, in0=vbS, scalar=1.0, in1=eqB,
                        op0=Alu.mult, op1=Alu.mult,
                        accum_out=REC[asg][:, bh * 2 + 1 : bh * 2 + 2],
                    )

                # ---- reorder ----------------------------------------------
                dif = pwork.tile([128, S - 1, D], f32, tag="dif")
                nc.gpsimd.tensor_tensor(
                    out=dif, in0=ah_n[:, 1:S, :], in1=ah_n[:, 0 : S - 1, :],
                    op=Alu.subtract,
                )
                dsq = pwork.tile([128, S - 1, D], f32, tag="dsq")
                nc.scalar.activation(
                    out=dsq, in_=dif, func=Act.Square,
                    accum_out=REO[:, bh : bh + 1],
                )

            # ---- final combine -------------------------------------------
            fin = state.tile([128, 8], f32, tag="fin")
            nc.vector.tensor_reduce(out=fin[:, 0:1], in_=REC[0], axis=AX.X, op=Alu.add)
            nc.vector.tensor_reduce(out=fin[:, 1:2], in_=REC[1], axis=AX.X, op=Alu.add)
            nc.vector.tensor_reduce(out=fin[:, 2:3], in_=KLA, axis=AX.X, op=Alu.add)
            nc.vector.tensor_reduce(out=fin[:, 3:4], in_=KLB, axis=AX.X, op=Alu.add)
            nc.vector.tensor_reduce(out=fin[:, 4:5], in_=KLC, axis=AX.X, op=Alu.add)
            nc.vector.tensor_reduce(out=fin[:, 5:6], in_=ENT, axis=AX.X, op=Alu.add)
            nc.vector.tensor_reduce(out=fin[:, 6:7], in_=REO, axis=AX.X, op=Alu.add)

            # total = 0.5*rec1 + rec2 + reorder - ent
            #         - (BETA/2) * (S*D*NBH + klc - kla - klb)
            acc = state.tile([128, 1], f32, tag="acc")
            tmp = state.tile([128, 1], f32, tag="tmp")
            nc.vector.tensor_scalar(
                out=acc, in0=fin[:, 0:1], scalar1=0.5, scalar2=None, op0=Alu.mult
            )
            nc.vector.tensor_tensor(out=acc, in0=acc, in1=fin[:, 1:2], op=Alu.add)
            nc.vector.tensor_tensor(out=acc, in0=acc, in1=fin[:, 6:7], op=Alu.add)
            nc.vector.tensor_tensor(out=acc, in0=acc, in1=fin[:, 5:6], op=Alu.subtract)
            nc.vector.tensor_scalar(
                out=tmp, in0=fin[:, 4:5], scalar1=float(S * D * NBH), scalar2=None,
                op0=Alu.add,
            )
            nc.vector.tensor_tensor(out=tmp, in0=tmp, in1=fin[:, 2:3], op=Alu.subtract)
            nc.vector.tensor_tensor(out=tmp, in0=tmp, in1=fin[:, 3:4], op=Alu.subtract)
            nc.vector.scalar_tensor_tensor(
                out=acc, in0=tmp, scalar=-BETA / 2.0, in1=acc,
                op0=Alu.mult, op1=Alu.add,
            )

            pfin = ptav.tile([1, 1], f32, tag="tav", name="pfin")
            nc.tensor.matmul(pfin, acc, ones_c)
            osb = state.tile([1, 1], f32, tag="osb")
            nc.scalar.copy(out=osb, in_=pfin)
            nc.sync.dma_start(out=out_d, in_=osb)

    nc.compile()
    return nc


def _get_nc():
    if "nc" not in _nc_cache:
        _nc_cache["nc"] = build_bass()
    return _nc_cache["nc"]


def kernel(ai, a_hat, mu_q, logvar_q, o, learned_mask, gamma):
    from concourse.bass_utils import run_bass_kernel_spmd

    nc = _get_nc()
    full = {
        "ai": np.ascontiguousarray(ai, np.float32),
        "a_hat": np.ascontiguousarray(a_hat, np.float32),
        "mu_q": np.ascontiguousarray(mu_q, np.float32),
        "logvar_q": np.ascontiguousarray(logvar_q, np.float32),
        "o": np.ascontiguousarray(o, np.float32),
        "learned_mask": np.ascontiguousarray(learned_mask, np.float32),
    }
    gam = np.ascontiguousarray(gamma, np.float32)
    in_maps = []
    for c in range(N_CORES):
        sl = slice(c * B, (c + 1) * B)
        m = {k: v[sl] for k, v in full.items()}
        m["gamma"] = gam
        in_maps.append(m)

    res = run_bass_kernel_spmd(
        nc, in_maps, core_ids=list(range(N_CORES)),
        trace=bool(int(os.environ.get("KBENCH_TRACE", "0"))),
    )
    total = np.float32(0.0)
    for r in res.results:
        total += np.float32(r["out"][0, 0])
    if res.exec_time_ns is not None:
        kernel.last_exec_time_ns = res.exec_time_ns
    kernel.last_results = res
    return np.asarray(total, dtype=np.float32)


kernel.last_exec_time_ns = None
kernel.last_results = None

